# revision 13
# baseline (speedup 1.0000x reference)
"""Trainium2 Bass kernel for the ConvFeatureExtractor problem.

Reference computation (all f32):
    matches[f, i] = sum_j kmer_params[f, kmer_idcs[i, j], j]      # (F, M)
    probs = softmax(matches / temperature, axis=1)                # over M
    pooled = freq @ probs.T                                       # (B, F)
    profile = pooled / pooled.sum(axis=1, keepdims=True)

Shapes: B=1024, M=4096 (=4^6 kmers), F=8192 filters, K=6, 4 bases.

Kernel strategy (8 NeuronCores, filter-sharded: FL = F/8 = 1024 per core):
  * matches^T = onehot(M, 24) @ params_flat^T(24, FL) as a K=24 bf16 matmul.
    Params are pre-shifted on host by (rowmax_f - margin*T)/K where
    rowmax_f = sum_j max_c kp[f,c,j] upper-bounds every row of matches, so
    exp((matches - rowmax + margin)/T) <= e^margin stays in fp8 e4m3 range.
    The per-filter shift cancels in the softmax normalization.
  * E = exp(matches'/T) written as fp8 e4m3 straight out of the ACT engine.
  * U = freq @ E^T with fp8 DoubleRow matmuls: each instruction contracts
    two 128-row k-tiles (rhs/lhsT carry a [2] pair dim), 0.5 cycles/row.
  * Z[f] = sum_i E[i, f] via an extra all-ones DoubleRow accumulator row
    (ones(128).T @ E broadcasts the column sums to every partition).
  * Evacuate U with DVE tensor_tensor_reduce: U_sb = psum * (1/Z) and
    accum_out gives the row-sums s[b] in the same pass.
  * 4KB AllReduce of s over the 8 cores (a dummy warmup collective is
    issued at kernel start to absorb the CC-stream spin-up latency);
    profile = pooled * (1/s).
Each core returns its (B, FL) f32 slice; host concatenates along F.
"""

import os

import numpy as np
import ml_dtypes

import concourse.bass as bass  # noqa: F401  (AP types come through tile/bacc)
import concourse.tile as tile
from concourse import bacc, mybir
from concourse.bass_utils import run_bass_kernel_spmd

NCORES = 8
B = 1024           # batch
M = 4096           # 4^6 kmers
F = 8192           # filters
KMER = 6           # kmer length
NBASE = 4
KK = NBASE * KMER  # 24 flattened (base, position)
FL = F // NCORES   # 1024 filters per core

MT = M // 128      # 32 contraction tiles
NP2 = MT // 2      # 16 DoubleRow pair steps
BT = B // 128      # 8 batch tiles
FC = 512           # psum free chunk
MARGIN = 5.0       # exp(margin) = 148 < 240 (fp8 e4m3 max normal)
# 1: freq quantized to one fp8 term; 2: fp8 + fp8 residual (more accurate,
# doubles the pooled-matmul work)
FREQ_TERMS = int(os.environ.get("KERNEL_FREQ_TERMS", "1"))

BF16 = mybir.dt.bfloat16
F8 = mybir.dt.float8e4
F32 = mybir.dt.float32
AFT = mybir.ActivationFunctionType
ALU = mybir.AluOpType
DR = mybir.MatmulPerfMode.DoubleRow

_CACHE: dict = {}


def _body(tc, freqT, freqTb, onehotT, paramsT, tempr, out):
    nc = tc.nc
    with (
        tc.tile_pool(name="res", bufs=1) as res,
        tc.tile_pool(name="pm", bufs=2, space="PSUM") as pm,
        tc.tile_pool(name="pacc", bufs=1, space="PSUM") as pacc,
        tc.tile_pool(name="dram", bufs=1, space="DRAM") as dram,
        tc.tile_pool(name="outp", bufs=4) as outp,
    ):
        # ---------- warmup collective: hurts (CC stream serializes) --------
        if os.environ.get("KERNEL_CC_WARMUP") and not os.environ.get(
                "KERNEL_NO_COLLECTIVE"):
            w_sb = res.tile([128, 1], F32)
            nc.vector.memset(w_sb[:], 0.0)
            w_in = dram.tile([128, 1], F32)
            w_out = dram.tile([128, 1], F32, addr_space="Shared")
            nc.sync.dma_start(w_in[:], w_sb[:])
            nc.gpsimd.collective_compute(
                "AllReduce", ALU.add,
                replica_groups=[list(range(NCORES))],
                ins=[w_in.opt()], outs=[w_out.opt()])

        # ---------- small inputs / constants (sync queue) ----------
        # order matters: few-partition DMAs are slow (~17 GB/s) and the sync
        # ring is FIFO, so load the smallest/most-urgent first and chunk oh
        t_sb = res.tile([128, 1], F32)       # T replicated on host to (128,1)
        nc.sync.dma_start(t_sb[:], tempr[:])
        par_sb = res.tile([KK, FL], BF16)
        nc.sync.dma_start(par_sb[:], paramsT[:])
        oh_sb = res.tile([KK, M], BF16)
        for c in range(4):
            nc.sync.dma_start(oh_sb[:, c * 1024:(c + 1) * 1024],
                              onehotT[:, c * 1024:(c + 1) * 1024])
        invt_bc = res.tile([128, 1], F32)    # per-partition 1/T activation scale
        nc.vector.reciprocal(invt_bc[:], t_sb[:])
        ones8 = res.tile([128, 2, 128], F8)  # DoubleRow all-ones lhsT pair
        nc.vector.memset(ones8[:], 1.0)

        # ---------- PE clock warmup ----------
        # the HAM clock gate keeps the PE at 1.2 GHz until it sees ~3.4us of
        # sustained activity; burn tiny matmuls while the input DMAs land so
        # the real work starts at full clock (results are never read)
        ones_bf = res.tile([128, 128], BF16)
        nc.vector.memset(ones_bf[:], 1.0)
        dummy_ps = pm.tile([128, 1024], F32, tag="pm")
        for _ in range(220):
            nc.tensor.matmul(dummy_ps[:, 0:128], lhsT=ones_bf[:],
                             rhs=ones_bf[:], start=True, stop=True)

        # ---------- freq (fp8, pre-tiled on host) on the ACT dma ring ------
        # chunked so the first k-pairs land early and the small sync-ring
        # DMAs above don't starve behind a single 4MB HBM burst
        freq_sb = res.tile([128, MT, B], F8)
        for c in range(4):
            nc.scalar.dma_start(freq_sb[:, c * 8:(c + 1) * 8, :],
                                freqT[:, c * 8:(c + 1) * 8, :])
        if FREQ_TERMS == 2:
            freqb_sb = res.tile([128, MT, B], F8)
            for c in range(4):
                nc.scalar.dma_start(freqb_sb[:, c * 8:(c + 1) * 8, :],
                                    freqTb[:, c * 8:(c + 1) * 8, :])

        E_sb = res.tile([128, MT, FL], F8)
        U_sb = res.tile([128, BT, FL], F32)
        invz = res.tile([128, FL], F32)
        s_col = res.tile([128, BT], F32)

        # persistent PSUM accumulators: Z row + batch-tile 0
        pz = pacc.tile([128, 1024], F32)
        pb0 = pacc.tile([128, 1024], F32)

        # ---------- phase A: matches -> E (fp8), Z and b0 accumulate ------
        # software-pipelined: DR accumulation for pair k2-1 is issued after
        # the bf16 matmuls of pair k2, so the PE never stalls on the ACT exp
        def dr_step(k2):
            first, last = (k2 == 0), (k2 == NP2 - 1)
            er0 = E_sb[:, 2 * k2:2 * k2 + 2, 0:FC]
            er1 = E_sb[:, 2 * k2:2 * k2 + 2, FC:2 * FC]
            nc.tensor.matmul(pz[:, 0:FC], lhsT=ones8[:], rhs=er0,
                             perf_mode=DR, start=first, stop=last)
            nc.tensor.matmul(pz[:, FC:2 * FC], lhsT=ones8[:], rhs=er1,
                             perf_mode=DR, start=first, stop=last)
            lw = freq_sb[:, 2 * k2:2 * k2 + 2, 0:128]
            stop0 = last and FREQ_TERMS == 1
            nc.tensor.matmul(pb0[:, 0:FC], lhsT=lw, rhs=er0,
                             perf_mode=DR, start=first, stop=stop0)
            nc.tensor.matmul(pb0[:, FC:2 * FC], lhsT=lw, rhs=er1,
                             perf_mode=DR, start=first, stop=stop0)
            if FREQ_TERMS == 2:
                lwb = freqb_sb[:, 2 * k2:2 * k2 + 2, 0:128]
                nc.tensor.matmul(pb0[:, 0:FC], lhsT=lwb, rhs=er0,
                                 perf_mode=DR, start=False, stop=last)
                nc.tensor.matmul(pb0[:, FC:2 * FC], lhsT=lwb, rhs=er1,
                                 perf_mode=DR, start=False, stop=last)

        for k2 in range(NP2 + 1):
            if k2 < NP2:
                for kk in (2 * k2, 2 * k2 + 1):
                    pm_t = pm.tile([128, 1024], F32, tag="pm")
                    lhs = oh_sb[:, kk * 128:(kk + 1) * 128]
                    nc.tensor.matmul(pm_t[:, 0:FC], lhsT=lhs,
                                     rhs=par_sb[:, 0:FC], start=True, stop=True)
                    nc.tensor.matmul(pm_t[:, FC:2 * FC], lhsT=lhs,
                                     rhs=par_sb[:, FC:2 * FC],
                                     start=True, stop=True)
                    nc.scalar.activation(E_sb[:, kk, :], pm_t[:],
                                         AFT.Exp, scale=invt_bc[:])
            if k2 >= 1:
                dr_step(k2 - 1)

        # ---------- 1/Z (broadcast on all partitions by the ones matmul) ---
        nc.vector.reciprocal(invz[:, 0:FC], pz[:, 0:FC])
        nc.vector.reciprocal(invz[:, FC:2 * FC], pz[:, FC:2 * FC])

        def evac(b, psum_t):
            # U_sb = psum * (1/Z), then row sums into s_col
            nc.vector.tensor_mul(U_sb[:, b, 0:FC], psum_t[:, 0:FC],
                                 invz[:, 0:FC])
            nc.vector.tensor_mul(U_sb[:, b, FC:2 * FC], psum_t[:, FC:2 * FC],
                                 invz[:, FC:2 * FC])
            nc.vector.reduce_sum(s_col[:, b:b + 1], U_sb[:, b, :],
                                 axis=mybir.AxisListType.X)

        evac(0, pb0)

        # ---------- phase B: batch tiles 1..7 ----------
        for b in range(1, BT):
            pu_t = pm.tile([128, 1024], F32, tag="pm")
            for k2 in range(NP2):
                first, last = (k2 == 0), (k2 == NP2 - 1)
                er0 = E_sb[:, 2 * k2:2 * k2 + 2, 0:FC]
                er1 = E_sb[:, 2 * k2:2 * k2 + 2, FC:2 * FC]
                lw = freq_sb[:, 2 * k2:2 * k2 + 2, b * 128:(b + 1) * 128]
                stop0 = last and FREQ_TERMS == 1
                nc.tensor.matmul(pu_t[:, 0:FC], lhsT=lw, rhs=er0,
                                 perf_mode=DR, start=first, stop=stop0)
                nc.tensor.matmul(pu_t[:, FC:2 * FC], lhsT=lw, rhs=er1,
                                 perf_mode=DR, start=first, stop=stop0)
                if FREQ_TERMS == 2:
                    lwb = freqb_sb[:, 2 * k2:2 * k2 + 2, b * 128:(b + 1) * 128]
                    nc.tensor.matmul(pu_t[:, 0:FC], lhsT=lwb, rhs=er0,
                                     perf_mode=DR, start=False, stop=last)
                    nc.tensor.matmul(pu_t[:, FC:2 * FC], lhsT=lwb, rhs=er1,
                                     perf_mode=DR, start=False, stop=last)
            evac(b, pu_t)

        # ---------- AllReduce of per-core rowsums (4KB) ----------
        s_sum = res.tile([128, BT], F32)
        if os.environ.get("KERNEL_NO_COLLECTIVE"):
            nc.vector.tensor_scalar_mul(s_sum[:], s_col[:], float(NCORES))
        else:
            s_in = dram.tile([128, BT], F32)
            s_out = dram.tile([128, BT], F32, addr_space="Shared")
            nc.sync.dma_start(s_in[:], s_col[:])
            nc.gpsimd.collective_compute(
                "AllReduce", ALU.add,
                replica_groups=[list(range(NCORES))],
                ins=[s_in.opt()], outs=[s_out.opt()])
            nc.sync.dma_start(s_sum[:], s_out[:])
        rinv = res.tile([128, BT], F32)
        nc.vector.reciprocal(rinv[:], s_sum[:])

        # ---------- profile = pooled * (1/s); write out (2 dma rings) ------
        for b in range(BT):
            prof = outp.tile([128, FL], F32, tag="prof")
            nc.vector.tensor_scalar_mul(prof[:], U_sb[:, b, :],
                                        rinv[:, b:b + 1])
            eng = nc.sync if b % 2 == 0 else nc.scalar
            eng.dma_start(out[b * 128:(b + 1) * 128, :], prof[:])


def _build_bass():
    nc = bacc.Bacc("TRN2", target_bir_lowering=False, debug=False,
                   num_devices=NCORES)
    freqT = nc.dram_tensor("freqT", [128, MT * B], F8, kind="ExternalInput").ap()
    freqT = freqT.rearrange("p (k b) -> p k b", k=MT)
    freqTb = None
    if FREQ_TERMS == 2:
        freqTb = nc.dram_tensor("freqTb", [128, MT * B], F8,
                                kind="ExternalInput").ap()
        freqTb = freqTb.rearrange("p (k b) -> p k b", k=MT)
    onehotT = nc.dram_tensor("onehotT", [KK, M], BF16, kind="ExternalInput").ap()
    paramsT = nc.dram_tensor("paramsT", [KK, FL], BF16, kind="ExternalInput").ap()
    tempr = nc.dram_tensor("tempr", [128, 1], F32, kind="ExternalInput").ap()
    out = nc.dram_tensor("out", [B, FL], F32, kind="ExternalOutput").ap()

    with tile.TileContext(nc) as tc:
        _body(tc, freqT, freqTb, onehotT, paramsT, tempr, out)
    nc.compile()
    return nc


def _get_nc():
    if "nc" not in _CACHE:
        _CACHE["nc"] = _build_bass()
    return _CACHE["nc"]


def _prepare_in_maps(freq, kmer_params, temperature, kmer_idcs):
    freq = np.asarray(freq, dtype=np.float32)            # (B, M)
    kp = np.asarray(kmer_params, dtype=np.float32)       # (F, 4, K)
    temp = np.asarray(temperature, dtype=np.float32).reshape(-1)[:1]
    idcs = np.asarray(kmer_idcs).astype(np.int64)        # (M, K)

    assert freq.shape == (B, M) and kp.shape == (F, NBASE, KMER)
    assert idcs.shape == (M, KMER)

    # one-hot re-encoding of the index input: onehot[i, c*K + j] = 1 iff
    # kmer_idcs[i, j] == c   (params_flat[f, c*K + j] = kmer_params[f, c, j])
    onehot = np.zeros((M, NBASE, KMER), dtype=np.float32)
    onehot[np.arange(M)[:, None], idcs, np.arange(KMER)[None, :]] = 1.0
    onehotT = np.ascontiguousarray(
        onehot.reshape(M, KK).T).astype(ml_dtypes.bfloat16)

    # shift params by (rowmax - margin*T)/K: bounds exp(matches'/T) by
    # e^margin for any kmer set (rowmax_f = sum_j max_c is an upper bound);
    # the per-filter factor cancels in the softmax normalization
    tval = float(temp[0])
    rowmax = kp.max(axis=1).sum(axis=1)                  # (F,)
    kp_shift = kp - ((rowmax - MARGIN * tval) / KMER)[:, None, None]
    params_flat = kp_shift.reshape(F, KK)

    # freq^T tiled to the SBUF layout [128, MT, B] and quantized to fp8
    f8 = ml_dtypes.float8_e4m3
    ftile = np.ascontiguousarray(
        freq.T.reshape(MT, 128, B).transpose(1, 0, 2)).reshape(128, MT * B)
    freqT8 = ftile.astype(f8)
    if FREQ_TERMS == 2:
        freqT8b = (ftile - freqT8.astype(np.float32)).astype(f8)
    tempr = np.ascontiguousarray(np.broadcast_to(temp.reshape(1, 1), (128, 1)))

    in_maps = []
    for c in range(NCORES):
        paramsT_c = np.ascontiguousarray(
            params_flat[c * FL:(c + 1) * FL].T).astype(ml_dtypes.bfloat16)
        im = {
            "freqT": freqT8,
            "onehotT": onehotT,
            "paramsT": paramsT_c,
            "tempr": tempr,
        }
        if FREQ_TERMS == 2:
            im["freqTb"] = freqT8b
        in_maps.append(im)
    return in_maps


def _run(in_maps, trace=False):
    nc = _get_nc()
    return run_bass_kernel_spmd(nc, in_maps, list(range(NCORES)), trace=trace)


def kernel(freq, kmer_params, temperature, kmer_idcs):
    in_maps = _prepare_in_maps(freq, kmer_params, temperature, kmer_idcs)
    res = _run(in_maps,
               trace=os.environ.get("KERNEL_TRACE", "") not in ("", "0"))
    _CACHE["last_result"] = res
    return np.concatenate(
        [np.asarray(res.results[c]["out"], dtype=np.float32)
         for c in range(NCORES)], axis=1)


# revision 18
# speedup vs baseline: 1.1869x; 1.1869x over previous
"""Trainium2 Bass kernel for the ConvFeatureExtractor problem.

Reference computation (all f32):
    matches[f, i] = sum_j kmer_params[f, kmer_idcs[i, j], j]      # (F, M)
    probs = softmax(matches / temperature, axis=1)                # over M
    pooled = freq @ probs.T                                       # (B, F)
    profile = pooled / pooled.sum(axis=1, keepdims=True)

Shapes: B=1024, M=4096 (=4^6 kmers), F=8192 filters, K=6, 4 bases.

Kernel strategy (8 NeuronCores, filter-sharded: FL = F/8 = 1024 per core):
  * matches^T = onehot(M, 24) @ params_flat^T(24, FL) as a K=24 bf16 matmul.
    Params are pre-shifted on host by (rowmax_f - margin*T)/K where
    rowmax_f = sum_j max_c kp[f,c,j] upper-bounds every row of matches, so
    exp((matches - rowmax + margin)/T) <= e^margin stays in fp8 e4m3 range.
    The per-filter shift cancels in the softmax normalization.
  * E = exp(matches'/T) written as fp8 e4m3 straight out of the ACT engine.
  * U = freq @ E^T with fp8 DoubleRow matmuls: each instruction contracts
    two 128-row k-tiles (rhs/lhsT carry a [2] pair dim), 0.5 cycles/row.
  * Z[f] = sum_i E[i, f] via an extra all-ones DoubleRow accumulator row
    (ones(128).T @ E broadcasts the column sums to every partition).
  * Evacuate U with DVE tensor_tensor_reduce: U_sb = psum * (1/Z) and
    accum_out gives the row-sums s[b] in the same pass.
  * 4KB AllReduce of s over the 8 cores (a dummy warmup collective is
    issued at kernel start to absorb the CC-stream spin-up latency);
    profile = pooled * (1/s).
Each core returns its (B, FL) f32 slice; host concatenates along F.
"""

import os

import numpy as np
import ml_dtypes

import concourse.bass as bass  # noqa: F401  (AP types come through tile/bacc)
import concourse.tile as tile
from concourse import bacc, mybir
from concourse.bass_utils import run_bass_kernel_spmd

NCORES = 8
B = 1024           # batch
M = 4096           # 4^6 kmers
F = 8192           # filters
KMER = 6           # kmer length
NBASE = 4
KK = NBASE * KMER  # 24 flattened (base, position)
FL = F // NCORES   # 1024 filters per core

MT = M // 128      # 32 contraction tiles
NP2 = MT // 2      # 16 DoubleRow pair steps
BT = B // 128      # 8 batch tiles
FC = 512           # psum free chunk
MARGIN = 5.0       # exp(margin) = 148 < 240 (fp8 e4m3 max normal)
# 1: freq quantized to one fp8 term; 2: fp8 + fp8 residual (more accurate,
# doubles the pooled-matmul work)
FREQ_TERMS = int(os.environ.get("KERNEL_FREQ_TERMS", "1"))

BF16 = mybir.dt.bfloat16
F8 = mybir.dt.float8e4
F32 = mybir.dt.float32
AFT = mybir.ActivationFunctionType
ALU = mybir.AluOpType
DR = mybir.MatmulPerfMode.DoubleRow

_CACHE: dict = {}


def _body(tc, freqT, freqTb, onehotT, paramsT, tempr, out):
    nc = tc.nc
    with (
        tc.tile_pool(name="res", bufs=1) as res,
        tc.tile_pool(name="pm", bufs=2, space="PSUM") as pm,
        tc.tile_pool(name="pacc", bufs=1, space="PSUM") as pacc,
        tc.tile_pool(name="dram", bufs=1, space="DRAM") as dram,
        tc.tile_pool(name="outp", bufs=4) as outp,
    ):
        # ---------- warmup collective: hurts (CC stream serializes) --------
        if os.environ.get("KERNEL_CC_WARMUP") and not os.environ.get(
                "KERNEL_NO_COLLECTIVE"):
            w_sb = res.tile([128, 1], F32)
            nc.vector.memset(w_sb[:], 0.0)
            w_in = dram.tile([128, 1], F32)
            w_out = dram.tile([128, 1], F32, addr_space="Shared")
            nc.sync.dma_start(w_in[:], w_sb[:])
            nc.gpsimd.collective_compute(
                "AllReduce", ALU.add,
                replica_groups=[list(range(NCORES))],
                ins=[w_in.opt()], outs=[w_out.opt()])

        # ---------- small inputs / constants (sync queue) ----------
        # order matters: few-partition DMAs are slow (~17 GB/s) and the sync
        # ring is FIFO, so load the smallest/most-urgent first and chunk oh
        t_sb = res.tile([128, 1], F32)       # T replicated on host to (128,1)
        nc.sync.dma_start(t_sb[:], tempr[:])
        par_sb = res.tile([KK, FL], BF16)
        nc.sync.dma_start(par_sb[:], paramsT[:])
        oh_sb = res.tile([KK, M], BF16)
        for c in range(4):
            nc.sync.dma_start(oh_sb[:, c * 1024:(c + 1) * 1024],
                              onehotT[:, c * 1024:(c + 1) * 1024])
        invt_bc = res.tile([128, 1], F32)    # per-partition 1/T activation scale
        nc.vector.reciprocal(invt_bc[:], t_sb[:])
        ones8 = res.tile([128, 2, 128], F8)  # DoubleRow all-ones lhsT pair
        nc.vector.memset(ones8[:], 1.0)

        # persistent PSUM accumulators: Z row + batch-tile 0
        pz = pacc.tile([128, 1024], F32)
        pb0 = pacc.tile([128, 1024], F32)

        # ---------- PE clock warmup ----------
        # the HAM clock gate keeps the PE at 1.2 GHz unless it sees ~3.4us of
        # sustained activity, and re-throttles on idle windows; burn tiny
        # matmuls while the input DMAs land, and pad phase A's dependency
        # stalls below so the clock stays warm (results are never read)
        ones_bf = res.tile([128, 128], BF16)
        nc.vector.memset(ones_bf[:], 1.0)
        zeros_bf = res.tile([128, 128], BF16)
        nc.vector.memset(zeros_bf[:], 0.0)
        junk = res.tile([128, 1], F32)
        # touch the exp table now: the first ACTIVATE otherwise stalls the
        # pipeline ~2.7us on ACT_TABLE_LOAD mid-phase
        nc.scalar.activation(junk[:], t_sb[:], AFT.Exp)

        def pad(n):
            # ones.T @ zeros accumulates +0 into pz: pure PE-activity filler
            # with no PSUM footprint of its own and no data hazards (the
            # first real pz matmul has start=True and resets everything)
            for _ in range(n):
                nc.tensor.matmul(pz[:, 0:128], lhsT=ones_bf[:],
                                 rhs=zeros_bf[:], start=False, stop=False,
                                 skip_group_check=True)

        pad(80)

        # ---------- freq (fp8, pre-tiled on host) on the ACT dma ring ------
        # chunked so the first k-pairs land early and the small sync-ring
        # DMAs above don't starve behind a single 4MB HBM burst
        freq_sb = res.tile([128, MT, B], F8)
        for c in range(4):
            nc.scalar.dma_start(freq_sb[:, c * 8:(c + 1) * 8, :],
                                freqT[:, c * 8:(c + 1) * 8, :])
        if FREQ_TERMS == 2:
            freqb_sb = res.tile([128, MT, B], F8)
            for c in range(4):
                nc.scalar.dma_start(freqb_sb[:, c * 8:(c + 1) * 8, :],
                                    freqTb[:, c * 8:(c + 1) * 8, :])

        E_sb = res.tile([128, MT, FL], F8)
        U_sb = res.tile([128, BT, FL], F32)
        invz = res.tile([128, FL], F32)
        s_col = res.tile([128, BT], F32)

        # ---------- phase A: matches -> E (fp8), Z and b0 accumulate ------
        # software-pipelined: DR accumulation for pair k2-1 is issued after
        # the bf16 matmuls of pair k2, so the PE never stalls on the ACT exp
        def dr_step(k2):
            first, last = (k2 == 0), (k2 == NP2 - 1)
            er0 = E_sb[:, 2 * k2:2 * k2 + 2, 0:FC]
            er1 = E_sb[:, 2 * k2:2 * k2 + 2, FC:2 * FC]
            nc.tensor.matmul(pz[:, 0:FC], lhsT=ones8[:], rhs=er0,
                             perf_mode=DR, start=first, stop=last)
            nc.tensor.matmul(pz[:, FC:2 * FC], lhsT=ones8[:], rhs=er1,
                             perf_mode=DR, start=first, stop=last)
            lw = freq_sb[:, 2 * k2:2 * k2 + 2, 0:128]
            stop0 = last and FREQ_TERMS == 1
            nc.tensor.matmul(pb0[:, 0:FC], lhsT=lw, rhs=er0,
                             perf_mode=DR, start=first, stop=stop0)
            nc.tensor.matmul(pb0[:, FC:2 * FC], lhsT=lw, rhs=er1,
                             perf_mode=DR, start=first, stop=stop0)
            if FREQ_TERMS == 2:
                lwb = freqb_sb[:, 2 * k2:2 * k2 + 2, 0:128]
                nc.tensor.matmul(pb0[:, 0:FC], lhsT=lwb, rhs=er0,
                                 perf_mode=DR, start=False, stop=last)
                nc.tensor.matmul(pb0[:, FC:2 * FC], lhsT=lwb, rhs=er1,
                                 perf_mode=DR, start=False, stop=last)

        for k2 in range(NP2 + 1):
            if k2 < NP2:
                for kk in (2 * k2, 2 * k2 + 1):
                    pad(8)
                    pm_t = pm.tile([128, 1024], F32, tag="pm")
                    lhs = oh_sb[:, kk * 128:(kk + 1) * 128]
                    nc.tensor.matmul(pm_t[:, 0:FC], lhsT=lhs,
                                     rhs=par_sb[:, 0:FC], start=True, stop=True)
                    nc.tensor.matmul(pm_t[:, FC:2 * FC], lhsT=lhs,
                                     rhs=par_sb[:, FC:2 * FC],
                                     start=True, stop=True)
                    nc.scalar.activation(E_sb[:, kk, :], pm_t[:],
                                         AFT.Exp, scale=invt_bc[:])
            if k2 >= 1:
                pad(4)
                dr_step(k2 - 1)

        # ---------- 1/Z (broadcast on all partitions by the ones matmul) ---
        nc.vector.reciprocal(invz[:, 0:FC], pz[:, 0:FC])
        nc.vector.reciprocal(invz[:, FC:2 * FC], pz[:, FC:2 * FC])

        def evac(b, psum_t):
            # U_sb = psum * (1/Z), then row sums into s_col
            nc.vector.tensor_mul(U_sb[:, b, 0:FC], psum_t[:, 0:FC],
                                 invz[:, 0:FC])
            nc.vector.tensor_mul(U_sb[:, b, FC:2 * FC], psum_t[:, FC:2 * FC],
                                 invz[:, FC:2 * FC])
            nc.vector.reduce_sum(s_col[:, b:b + 1], U_sb[:, b, :],
                                 axis=mybir.AxisListType.X)

        evac(0, pb0)

        # ---------- phase B: batch tiles 1..7 ----------
        for b in range(1, BT):
            pu_t = pm.tile([128, 1024], F32, tag="pm")
            for k2 in range(NP2):
                first, last = (k2 == 0), (k2 == NP2 - 1)
                er0 = E_sb[:, 2 * k2:2 * k2 + 2, 0:FC]
                er1 = E_sb[:, 2 * k2:2 * k2 + 2, FC:2 * FC]
                lw = freq_sb[:, 2 * k2:2 * k2 + 2, b * 128:(b + 1) * 128]
                stop0 = last and FREQ_TERMS == 1
                nc.tensor.matmul(pu_t[:, 0:FC], lhsT=lw, rhs=er0,
                                 perf_mode=DR, start=first, stop=stop0)
                nc.tensor.matmul(pu_t[:, FC:2 * FC], lhsT=lw, rhs=er1,
                                 perf_mode=DR, start=first, stop=stop0)
                if FREQ_TERMS == 2:
                    lwb = freqb_sb[:, 2 * k2:2 * k2 + 2, b * 128:(b + 1) * 128]
                    nc.tensor.matmul(pu_t[:, 0:FC], lhsT=lwb, rhs=er0,
                                     perf_mode=DR, start=False, stop=last)
                    nc.tensor.matmul(pu_t[:, FC:2 * FC], lhsT=lwb, rhs=er1,
                                     perf_mode=DR, start=False, stop=last)
            evac(b, pu_t)

        # ---------- AllReduce of per-core rowsums (4KB) ----------
        s_sum = res.tile([128, BT], F32)
        if os.environ.get("KERNEL_NO_COLLECTIVE"):
            nc.vector.tensor_scalar_mul(s_sum[:], s_col[:], float(NCORES))
        else:
            s_in = dram.tile([128, BT], F32)
            s_out = dram.tile([128, BT], F32, addr_space="Shared")
            nc.sync.dma_start(s_in[:], s_col[:])
            nc.gpsimd.collective_compute(
                "AllReduce", ALU.add,
                replica_groups=[list(range(NCORES))],
                ins=[s_in.opt()], outs=[s_out.opt()])
            nc.sync.dma_start(s_sum[:], s_out[:])
        rinv = res.tile([128, BT], F32)
        nc.vector.reciprocal(rinv[:], s_sum[:])

        # ---------- profile = pooled * (1/s); write out (2 dma rings) ------
        for b in range(BT):
            prof = outp.tile([128, FL], F32, tag="prof")
            nc.vector.tensor_scalar_mul(prof[:], U_sb[:, b, :],
                                        rinv[:, b:b + 1])
            eng = nc.sync if b % 2 == 0 else nc.scalar
            eng.dma_start(out[b * 128:(b + 1) * 128, :], prof[:])


def _build_bass():
    nc = bacc.Bacc("TRN2", target_bir_lowering=False, debug=False,
                   num_devices=NCORES)
    freqT = nc.dram_tensor("freqT", [128, MT * B], F8, kind="ExternalInput").ap()
    freqT = freqT.rearrange("p (k b) -> p k b", k=MT)
    freqTb = None
    if FREQ_TERMS == 2:
        freqTb = nc.dram_tensor("freqTb", [128, MT * B], F8,
                                kind="ExternalInput").ap()
        freqTb = freqTb.rearrange("p (k b) -> p k b", k=MT)
    onehotT = nc.dram_tensor("onehotT", [KK, M], BF16, kind="ExternalInput").ap()
    paramsT = nc.dram_tensor("paramsT", [KK, FL], BF16, kind="ExternalInput").ap()
    tempr = nc.dram_tensor("tempr", [128, 1], F32, kind="ExternalInput").ap()
    out = nc.dram_tensor("out", [B, FL], F32, kind="ExternalOutput").ap()

    with tile.TileContext(nc) as tc:
        _body(tc, freqT, freqTb, onehotT, paramsT, tempr, out)
    nc.compile()
    return nc


def _get_nc():
    if "nc" not in _CACHE:
        _CACHE["nc"] = _build_bass()
    return _CACHE["nc"]


def _prepare_in_maps(freq, kmer_params, temperature, kmer_idcs):
    freq = np.asarray(freq, dtype=np.float32)            # (B, M)
    kp = np.asarray(kmer_params, dtype=np.float32)       # (F, 4, K)
    temp = np.asarray(temperature, dtype=np.float32).reshape(-1)[:1]
    idcs = np.asarray(kmer_idcs).astype(np.int64)        # (M, K)

    assert freq.shape == (B, M) and kp.shape == (F, NBASE, KMER)
    assert idcs.shape == (M, KMER)

    # one-hot re-encoding of the index input: onehot[i, c*K + j] = 1 iff
    # kmer_idcs[i, j] == c   (params_flat[f, c*K + j] = kmer_params[f, c, j])
    onehot = np.zeros((M, NBASE, KMER), dtype=np.float32)
    onehot[np.arange(M)[:, None], idcs, np.arange(KMER)[None, :]] = 1.0
    onehotT = np.ascontiguousarray(
        onehot.reshape(M, KK).T).astype(ml_dtypes.bfloat16)

    # shift params by (rowmax - margin*T)/K: bounds exp(matches'/T) by
    # e^margin for any kmer set (rowmax_f = sum_j max_c is an upper bound);
    # the per-filter factor cancels in the softmax normalization
    tval = float(temp[0])
    rowmax = kp.max(axis=1).sum(axis=1)                  # (F,)
    kp_shift = kp - ((rowmax - MARGIN * tval) / KMER)[:, None, None]
    params_flat = kp_shift.reshape(F, KK)

    # freq^T tiled to the SBUF layout [128, MT, B] and quantized to fp8
    f8 = ml_dtypes.float8_e4m3
    ftile = np.ascontiguousarray(
        freq.T.reshape(MT, 128, B).transpose(1, 0, 2)).reshape(128, MT * B)
    freqT8 = ftile.astype(f8)
    if FREQ_TERMS == 2:
        freqT8b = (ftile - freqT8.astype(np.float32)).astype(f8)
    tempr = np.ascontiguousarray(np.broadcast_to(temp.reshape(1, 1), (128, 1)))

    in_maps = []
    for c in range(NCORES):
        paramsT_c = np.ascontiguousarray(
            params_flat[c * FL:(c + 1) * FL].T).astype(ml_dtypes.bfloat16)
        im = {
            "freqT": freqT8,
            "onehotT": onehotT,
            "paramsT": paramsT_c,
            "tempr": tempr,
        }
        if FREQ_TERMS == 2:
            im["freqTb"] = freqT8b
        in_maps.append(im)
    return in_maps


def _run(in_maps, trace=False):
    nc = _get_nc()
    return run_bass_kernel_spmd(nc, in_maps, list(range(NCORES)), trace=trace)


def kernel(freq, kmer_params, temperature, kmer_idcs):
    in_maps = _prepare_in_maps(freq, kmer_params, temperature, kmer_idcs)
    res = _run(in_maps,
               trace=os.environ.get("KERNEL_TRACE", "") not in ("", "0"))
    _CACHE["last_result"] = res
    return np.concatenate(
        [np.asarray(res.results[c]["out"], dtype=np.float32)
         for c in range(NCORES)], axis=1)
